# revision 9
# baseline (speedup 1.0000x reference)
"""ConcatRelationModule Bass kernel for 8 trn2 NeuronCores — v11.

Per edge e in [0, 16383):
    x      = concat(inputs[heads[e], 0, :], inputs[e + 1, 1, :])     # [512]
    h      = tanh(concat(x @ W_FOH, x @ W_FOM) + b1)                 # [1024]
    h2     = tanh(h @ W2 + b2)                                       # [256]
    out[e] = h2 @ W3 + b3                                            # [E, 64]

v11 = v10 restructured around the measured bottlenecks:
  - warm-up matmuls gated only on a DVE memset (not the gpsimd queue),
    so they actually run during the DMA prologue and keep the HAM
    clock-gate warm (K=8/8) before real matmuls start
  - identity built on DVE; gpsimd queue = headsT DMA + the 16 indirect
    gathers, nothing else (gather descriptor-gen is the pacing item)
  - direct loads spread across sync/scalar/vector queues so descriptor
    generation isn't serialized on one engine
  - gather/xm pools sized one-buffer-per-group: no back-pressure on
    gather prefetch
  - L2/L3 lag the L1 pipeline by one group: w2's arrival is off the
    critical path and L2's last-kc matmul never waits on a fresh ACT
  - tanh ACT table preloaded by a dummy activation at t~7us
  - output stored bf16 (converted + biased on host): halves output DMA
  - groups ordered 256,512,512,512,128,128: small tail
"""

import os

import numpy as np
import ml_dtypes

import concourse.bass as bass
import concourse.bacc as bacc
import concourse.mybir as mybir
import concourse.tile as tile
from concourse.bass import IndirectOffsetOnAxis
from concourse.bass_utils import run_bass_kernel_spmd

N_TOKENS = 16384
LD = 256
HID = 512
HID2 = 256
NREL = 64
NCORES = 8
E = N_TOKENS - 1
EPC = N_TOKENS // NCORES  # 2048
P = 128
SUB = EPC // P            # 16
N_WARMUP = 40

GROUPS = [(0, 256), (256, 512), (768, 512), (1280, 512), (1792, 128),
          (1920, 128)]

LAST_RESULTS = None
_CACHE = {}


def _build():
    bf16 = mybir.dt.bfloat16
    f32 = mybir.dt.float32

    nc = bacc.Bacc()
    fwd = nc.declare_dram_parameter("fwd", [N_TOKENS, LD], bf16, isOutput=False)
    bwdT = nc.declare_dram_parameter("bwdT", [P, 2, EPC], bf16, isOutput=False)
    headsT = nc.declare_dram_parameter(
        "headsT", [P, SUB], mybir.dt.int32, isOutput=False)
    w1 = nc.declare_dram_parameter("w1", [2 * LD, 2 * HID], bf16, isOutput=False)
    w2p = nc.declare_dram_parameter("w2p", [P, 2, 8, HID2 // 2], bf16,
                                    isOutput=False)
    w3p = nc.declare_dram_parameter("w3p", [P, 2, NREL], bf16, isOutput=False)
    bpack = nc.declare_dram_parameter("bpack", [P, 10], f32, isOutput=False)
    identw = nc.declare_dram_parameter("identw", [P, P], bf16, isOutput=False)
    outT = nc.declare_dram_parameter("outT", [NREL, EPC], bf16, isOutput=True)

    Tanh = mybir.ActivationFunctionType.Tanh
    NG = len(GROUPS)

    with tile.TileContext(nc) as tc:
        with (
            tc.tile_pool(name="const", bufs=1) as const_pool,
            tc.tile_pool(name="xh", bufs=6) as xh_pool,
            tc.tile_pool(name="xm", bufs=6) as xm_pool,
            tc.tile_pool(name="xT", bufs=6) as xT_pool,
            tc.tile_pool(name="h1", bufs=16) as h1_pool,
            tc.tile_pool(name="h2", bufs=4) as h2_pool,
            tc.tile_pool(name="outs", bufs=3) as out_pool,
            tc.tile_pool(name="pt", bufs=3, space="PSUM") as pt_pool,
            tc.tile_pool(name="ph", bufs=3, space="PSUM") as ph_pool,
            tc.tile_pool(name="pj", bufs=2, space="PSUM") as pj_pool,
        ):
            # --- prologue: per-engine queue order is emission order ---
            # DVE: warm-up scratch first (gates the PE warm-ups only)
            warm_sb = const_pool.tile([P, 512], bf16)
            nc.vector.memset(warm_sb[:], 0)

            # scalar: bias pack load
            bp_sb = const_pool.tile([P, 10], f32)
            nc.scalar.dma_start(bp_sb[:], bpack[:])

            # gpsimd: headsT via its own queue (gathers follow on it)
            hT_sb = const_pool.tile([P, SUB], mybir.dt.int32)
            nc.gpsimd.dma_start(hT_sb[:], headsT[:])

            # PE warm-ups: keep HAM at K=8/8 through the whole prologue.
            # Output never read; ~107ns each cold, ~4us total coverage.
            wps = pt_pool.tile([P, P], f32, tag="pt", name="warmup")
            for _ in range(N_WARMUP):
                nc.tensor.matmul(
                    out=wps[:], lhsT=warm_sb[:, 0:P], rhs=warm_sb[:, 0:P],
                    start=True, stop=True,
                )

            # sync: w1 k-chunks, modifier halves (kc 2,3) first
            w1_sb = [const_pool.tile([P, 2 * HID], bf16, tag=f"w1_{kc}",
                                     name=f"w1_{kc}")
                     for kc in range(4)]
            for kc in (2, 3, 0, 1):
                nc.sync.dma_start(w1_sb[kc][:], w1[kc * P:(kc + 1) * P, :])

            xg_tiles = [None] * NG
            xm_tiles = [None] * NG

            def load_group(gi, xm_eng):
                start, size = GROUPS[gi]
                ns = size // P
                xh = xh_pool.tile([P, ns, LD], bf16, tag="xh", name=f"xh_{gi}")
                for s in range(ns):
                    t = start // P + s
                    nc.gpsimd.indirect_dma_start(
                        out=xh[:, s, :],
                        out_offset=None,
                        in_=fwd[:],
                        in_offset=IndirectOffsetOnAxis(ap=hT_sb[:, t:t + 1], axis=0),
                    )
                # pre-transposed modifier half: direct, feature-major
                xm = xm_pool.tile([P, 2, size], bf16, tag="xm", name=f"xm_{gi}")
                xm_eng.dma_start(xm[:], bwdT[:, :, start:start + size])
                xg_tiles[gi] = xh
                xm_tiles[gi] = xm

            load_group(0, nc.scalar)

            # identity for PE transposes: shipped from DRAM (keeps gpsimd
            # free for gather descriptor generation)
            ident = const_pool.tile([P, P], bf16)
            nc.scalar.dma_start(ident[:], identw[:])

            # scalar: dummy ACT preloads the tanh table during the prologue
            scratch_sb = const_pool.tile([P, 1], f32)
            nc.scalar.activation(
                out=scratch_sb[:], in_=warm_sb[:, 0:1], func=Tanh, bias=0.0,
            )

            load_group(1, nc.scalar)

            # sync: w2 in two jc halves (first L2 needs only jc0)
            w2_sb = const_pool.tile([P, 2, 8, HID2 // 2], bf16)
            for jc in range(2):
                nc.sync.dma_start(w2_sb[:, jc], w2p[:, jc])

            load_group(2, nc.scalar)

            # scalar: w3 (tiny)
            w3_sb = const_pool.tile([P, 2, NREL], bf16)
            nc.scalar.dma_start(w3_sb[:], w3p[:])

            load_group(3, nc.scalar)
            load_group(4, nc.sync)
            load_group(5, nc.sync)

            xT_tiles = [None] * NG
            h1_tiles = [None] * NG
            h2_tiles = [None] * NG

            def emit_transpose(gi):
                start, size = GROUPS[gi]
                xh = xg_tiles[gi]
                xTs = []
                for kc in range(2):  # head half only
                    col = kc * P
                    pt = pt_pool.tile([P, size], bf16, tag="pt",
                                      name=f"pt_{gi}_{kc}")
                    for s in range(size // P):
                        nc.tensor.transpose(
                            pt[:, s * P:(s + 1) * P],
                            xh[:, s, col:col + P], ident[:])
                    xT = xT_pool.tile([P, size], bf16, tag="xT",
                                      name=f"xT_{gi}_{kc}")
                    nc.vector.tensor_copy(out=xT[:], in_=pt[:])
                    xTs.append(xT)
                xT_tiles[gi] = xTs

            def emit_l1(gi):
                start, size = GROUPS[gi]
                xTs = xT_tiles[gi]
                xm = xm_tiles[gi]
                h1s = []
                for hc in range(8):
                    ph = ph_pool.tile([P, size], f32, tag="ph",
                                      name=f"ph_{gi}_{hc}")
                    for i, kc in enumerate((2, 3, 0, 1)):
                        rhs = xm[:, kc - 2, :] if kc >= 2 else xTs[kc][:]
                        nc.tensor.matmul(
                            out=ph[:],
                            lhsT=w1_sb[kc][:, hc * P:(hc + 1) * P],
                            rhs=rhs,
                            start=(i == 0),
                            stop=(i == 3),
                        )
                    h1 = h1_pool.tile([P, size], bf16, tag="h1",
                                      name=f"h1_{gi}_{hc}")
                    nc.scalar.activation(
                        out=h1[:], in_=ph[:], func=Tanh,
                        bias=bp_sb[:, hc:hc + 1],
                    )
                    h1s.append(h1)
                h1_tiles[gi] = h1s

            def emit_l2l3(gi):
                start, size = GROUPS[gi]
                h1s = h1_tiles[gi]
                h2s = []
                for jc in range(2):
                    pj = pj_pool.tile([P, size], f32, tag="pj",
                                      name=f"pj_{gi}_{jc}")
                    for kc in range(8):
                        nc.tensor.matmul(
                            out=pj[:],
                            lhsT=w2_sb[:, jc, kc, :],
                            rhs=h1s[kc][:],
                            start=(kc == 0),
                            stop=(kc == 7),
                        )
                    h2 = h2_pool.tile([P, size], bf16, tag="h2",
                                      name=f"h2_{gi}_{jc}")
                    nc.scalar.activation(
                        out=h2[:], in_=pj[:], func=Tanh,
                        bias=bp_sb[:, 8 + jc:9 + jc],
                    )
                    h2s.append(h2)
                h2_tiles[gi] = h2s

                # L3 (b3 added on host)
                po = pt_pool.tile([NREL, size], f32, tag="pt", name=f"po_{gi}")
                for kc in range(2):
                    nc.tensor.matmul(
                        out=po[:],
                        lhsT=w3_sb[:, kc, :],
                        rhs=h2s[kc][:],
                        start=(kc == 0),
                        stop=(kc == 1),
                    )
                o = out_pool.tile([NREL, size], bf16, tag="o", name=f"o_{gi}")
                nc.vector.tensor_copy(out=o[:], in_=po[:])
                nc.sync.dma_start(outT[:, start:start + size], o[:])

            # --- pipeline: L2/L3 lag L1 by one group ---
            emit_transpose(0)
            for gi in range(NG):
                emit_l1(gi)
                if gi + 1 < NG:
                    emit_transpose(gi + 1)
                if gi >= 1:
                    emit_l2l3(gi - 1)
            emit_l2l3(NG - 1)

    nc.finalize()
    return nc


def _prep_inputs(inputs, rhidLayerFOH, rhidLayerFOM, rcatBias, rhid2Layer,
                 rhid2Bias, routLayer, routBias, heads):
    wdt = ml_dtypes.bfloat16
    inputs = np.asarray(inputs, dtype=np.float32)
    heads = np.asarray(heads)

    fwd = np.ascontiguousarray(inputs[:, 0, :]).astype(wdt)
    bwd_full = inputs[:, 1, :]
    mods_pad = np.concatenate(
        [np.arange(1, N_TOKENS), [N_TOKENS - 1]]).astype(np.int64)
    heads_pad = np.concatenate([heads.astype(np.int64), [0]]).astype(np.int32)

    w1 = np.ascontiguousarray(
        np.concatenate([np.asarray(rhidLayerFOH), np.asarray(rhidLayerFOM)],
                       axis=1)).astype(wdt)                      # [512, 1024]
    w2p = np.ascontiguousarray(
        np.asarray(rhid2Layer, dtype=np.float32)
        .reshape(8, P, 2, HID2 // 2).transpose(1, 2, 0, 3)).astype(wdt)
    w3p = np.ascontiguousarray(
        np.asarray(routLayer, dtype=np.float32)
        .reshape(2, P, NREL).transpose(1, 0, 2)).astype(wdt)
    b1 = np.asarray(rcatBias, dtype=np.float32).reshape(8, P).T
    b2 = np.asarray(rhid2Bias, dtype=np.float32).reshape(2, P).T
    bpack = np.ascontiguousarray(np.concatenate([b1, b2], axis=1))

    in_maps = []
    for c in range(NCORES):
        sl = slice(c * EPC, (c + 1) * EPC)
        bwd_c = bwd_full[mods_pad[sl]]                           # [2048, 256]
        bwdT_c = np.ascontiguousarray(
            bwd_c.T.reshape(2, P, EPC).transpose(1, 0, 2)).astype(wdt)
        headsT_c = np.ascontiguousarray(heads_pad[sl].reshape(SUB, P).T)
        in_maps.append({
            "fwd": fwd, "bwdT": bwdT_c, "headsT": headsT_c,
            "w1": w1, "w2p": w2p, "w3p": w3p, "bpack": bpack,
            "identw": np.eye(P, dtype=wdt),
        })
    return in_maps


def kernel(inputs, rhidLayerFOH, rhidLayerFOM, rcatBias, rhid2Layer, rhid2Bias,
           routLayer, routBias, heads):
    global LAST_RESULTS

    in_maps = _prep_inputs(inputs, rhidLayerFOH, rhidLayerFOM, rcatBias,
                           rhid2Layer, rhid2Bias, routLayer, routBias, heads)

    if "nc" not in _CACHE:
        _CACHE["nc"] = _build()
    nc = _CACHE["nc"]

    trace_dir = os.environ.get("KERNEL_TRACE_DIR") or None
    res = run_bass_kernel_spmd(nc, in_maps, list(range(NCORES)), tmpdir=trace_dir)
    LAST_RESULTS = res

    outT = np.concatenate(
        [np.asarray(r["outT"], dtype=np.float32) for r in res.results], axis=1)
    out = outT.T[:E] + np.asarray(routBias, dtype=np.float32)[None, :]
    return np.ascontiguousarray(out).astype(np.float32)


# revision 13
# speedup vs baseline: 1.0235x; 1.0235x over previous
"""ConcatRelationModule Bass kernel for 8 trn2 NeuronCores — v12.

Per edge e in [0, 16383):
    x      = concat(inputs[heads[e], 0, :], inputs[e + 1, 1, :])     # [512]
    h      = tanh(concat(x @ W_FOH, x @ W_FOM) + b1)                 # [1024]
    h2     = tanh(h @ W2 + b2)                                       # [256]
    out[e] = h2 @ W3 + b3                                            # [E, 64]

v12 core idea: the gather (gpsimd software descriptor generation,
~1.1us per 128 rows) is the pacing item early on, and any PE idle
>~3.4us re-throttles the HAM clock gate to half rate.  So L1 is split
into two passes per group of 256 edges:
  pass A: the modifier half (kc 2,3) — fed by a host-pretransposed
          direct DMA, no gather dependency
  pass B: the head half (kc 0,1) — closes the accumulation once the
          gather+transpose has landed
Eight hc accumulations stay open across the passes by packing hc pairs
into single PSUM banks ([P, 2, 256] f32 = one 2KB bank), so a group
needs 4 banks.  L2/L3 lag L1 by one group.  DMA deliveries are
priority-ordered: hT+gathers on gpsimd only; xm/bp/ident early on
scalar; w1 then w2 then cold tensors on sync.  Warm-up matmuls gated
only on a small DVE memset keep the PE (and HAM) busy from ~7us.
Output is stored bf16 and converted (+bias) on host.
"""

import os

import numpy as np
import ml_dtypes

import concourse.bass as bass
import concourse.bacc as bacc
import concourse.mybir as mybir
import concourse.tile as tile
from concourse.bass import IndirectOffsetOnAxis
from concourse.bass_utils import run_bass_kernel_spmd

N_TOKENS = 16384
LD = 256
HID = 512
HID2 = 256
NREL = 64
NCORES = 8
E = N_TOKENS - 1
EPC = N_TOKENS // NCORES  # 2048
P = 128
SUB = EPC // P            # 16
N_WARMUP = 30

GS = 256                  # group size (edges)
NG = EPC // GS            # 8 groups per core

LAST_RESULTS = None
_CACHE = {}


def _build():
    bf16 = mybir.dt.bfloat16
    f32 = mybir.dt.float32

    nc = bacc.Bacc()
    fwd = nc.declare_dram_parameter("fwd", [N_TOKENS, LD], bf16, isOutput=False)
    bwdT = nc.declare_dram_parameter("bwdT", [P, 2, EPC], bf16, isOutput=False)
    headsT = nc.declare_dram_parameter(
        "headsT", [P, SUB], mybir.dt.int32, isOutput=False)
    w1 = nc.declare_dram_parameter("w1", [2 * LD, 2 * HID], bf16, isOutput=False)
    w2p = nc.declare_dram_parameter("w2p", [P, 2, 8, HID2 // 2], bf16,
                                    isOutput=False)
    w3p = nc.declare_dram_parameter("w3p", [P, 2, NREL], bf16, isOutput=False)
    bpack = nc.declare_dram_parameter("bpack", [P, 10], f32, isOutput=False)
    identw = nc.declare_dram_parameter("identw", [P, P], bf16, isOutput=False)
    outT = nc.declare_dram_parameter("outT", [NREL, EPC], bf16, isOutput=True)

    Tanh = mybir.ActivationFunctionType.Tanh

    with tile.TileContext(nc) as tc:
        with (
            tc.tile_pool(name="const", bufs=1) as const_pool,
            tc.tile_pool(name="xh", bufs=NG) as xh_pool,
            tc.tile_pool(name="xm", bufs=NG) as xm_pool,
            tc.tile_pool(name="xT", bufs=4) as xT_pool,
            tc.tile_pool(name="h1", bufs=16) as h1_pool,
            tc.tile_pool(name="h2", bufs=4) as h2_pool,
            tc.tile_pool(name="outs", bufs=3) as out_pool,
            tc.tile_pool(name="pt", bufs=2, space="PSUM") as pt_pool,
            tc.tile_pool(name="ph", bufs=4, space="PSUM") as ph_pool,
            tc.tile_pool(name="pj", bufs=2, space="PSUM") as pj_pool,
        ):
            # --- prologue (per-engine emission order == queue order) ---
            # DVE: tiny warm-up scratch; only gates the PE warm-ups
            warm_sb = const_pool.tile([P, P], bf16)
            nc.vector.memset(warm_sb[:], 0)

            # gpsimd: headsT, then nothing but gathers
            hT_sb = const_pool.tile([P, SUB], mybir.dt.int32)
            nc.gpsimd.dma_start(hT_sb[:], headsT[:])

            # scalar: bias pack, first modifier slab, identity
            bp_sb = const_pool.tile([P, 10], f32)
            nc.scalar.dma_start(bp_sb[:], bpack[:])

            # PE warm-ups: keep HAM at K=8/8 through the whole prologue
            wps = pt_pool.tile([P, P], f32, tag="pt", name="warmup")
            for _ in range(N_WARMUP):
                nc.tensor.matmul(
                    out=wps[:], lhsT=warm_sb[:], rhs=warm_sb[:],
                    start=True, stop=True,
                )

            xg_tiles = [None] * NG
            xm_tiles = [None] * NG

            def load_xm(gi, eng):
                start = gi * GS
                xm = xm_pool.tile([P, 2, GS], bf16, tag="xm", name=f"xm_{gi}")
                eng.dma_start(xm[:], bwdT[:, :, start:start + GS])
                xm_tiles[gi] = xm

            def load_gather(gi):
                ns = GS // P
                xh = xh_pool.tile([P, ns, LD], bf16, tag="xh", name=f"xh_{gi}")
                for s in range(ns):
                    t = gi * ns + s
                    nc.gpsimd.indirect_dma_start(
                        out=xh[:, s, :],
                        out_offset=None,
                        in_=fwd[:],
                        in_offset=IndirectOffsetOnAxis(ap=hT_sb[:, t:t + 1], axis=0),
                    )
                xg_tiles[gi] = xh

            load_xm(0, nc.scalar)
            load_gather(0)

            ident = const_pool.tile([P, P], bf16)
            nc.scalar.dma_start(ident[:], identw[:])

            # scalar: dummy ACT preloads the tanh table during the prologue
            scratch_sb = const_pool.tile([P, 1], f32)
            nc.scalar.activation(
                out=scratch_sb[:], in_=warm_sb[:, 0:1], func=Tanh, bias=0.0,
            )

            # sync: w1 k-chunks, modifier halves (kc 2,3) first
            w1_sb = [const_pool.tile([P, 2 * HID], bf16, tag=f"w1_{kc}",
                                     name=f"w1_{kc}")
                     for kc in range(4)]
            for kc in (2, 3):
                nc.sync.dma_start(w1_sb[kc][:], w1[kc * P:(kc + 1) * P, :])

            load_xm(1, nc.scalar)
            load_gather(1)

            for kc in (0, 1):
                nc.sync.dma_start(w1_sb[kc][:], w1[kc * P:(kc + 1) * P, :])

            load_xm(2, nc.scalar)
            load_gather(2)

            # sync: w2 in two jc halves
            w2_sb = const_pool.tile([P, 2, 8, HID2 // 2], bf16)
            for jc in range(2):
                nc.sync.dma_start(w2_sb[:, jc], w2p[:, jc])

            load_xm(3, nc.scalar)
            load_gather(3)

            # sync: remaining cold tensors
            w3_sb = const_pool.tile([P, 2, NREL], bf16)
            nc.sync.dma_start(w3_sb[:], w3p[:])

            for gi in (4, 5, 6, 7):
                load_xm(gi, nc.sync)
                load_gather(gi)

            xT_tiles = [None] * NG
            h1_tiles = [None] * NG
            ph_tiles = [None] * NG

            def emit_transpose(gi):
                xh = xg_tiles[gi]
                xTs = []
                for kc in range(2):  # head half only
                    col = kc * P
                    pt = pt_pool.tile([P, GS], bf16, tag="pt",
                                      name=f"pt_{gi}_{kc}")
                    for s in range(GS // P):
                        nc.tensor.transpose(
                            pt[:, s * P:(s + 1) * P],
                            xh[:, s, col:col + P], ident[:])
                    xT = xT_pool.tile([P, GS], bf16, tag="xT",
                                      name=f"xT_{gi}_{kc}")
                    nc.vector.tensor_copy(out=xT[:], in_=pt[:])
                    xTs.append(xT)
                xT_tiles[gi] = xTs

            def emit_l1_a(gi):
                """Modifier half: open all 8 hc accumulations (kc 2,3)."""
                xm = xm_tiles[gi]
                xTs = xT_tiles[gi]
                phs = []
                for pair in range(4):
                    ph = ph_pool.tile([P, 2, GS], f32, tag="ph",
                                      name=f"ph_{gi}_{pair}")
                    phs.append(ph)
                    for sub in range(2):
                        hc = 2 * pair + sub
                        for i, kc in enumerate((2, 3, 0, 1)):
                            rhs = xm[:, kc - 2, :] if kc >= 2 else xTs[kc][:]
                            nc.tensor.matmul(
                                out=ph[:, sub, :],
                                lhsT=w1_sb[kc][:, hc * P:(hc + 1) * P],
                                rhs=rhs,
                                start=(i == 0),
                                stop=(i == 3),
                            )
                ph_tiles[gi] = phs

            def emit_l1_b(gi):
                """tanh the closed accumulations."""
                phs = ph_tiles[gi]
                h1s = []
                for hc in range(8):
                    h1 = h1_pool.tile([P, GS], bf16, tag="h1",
                                      name=f"h1_{gi}_{hc}")
                    nc.scalar.activation(
                        out=h1[:], in_=phs[hc // 2][:, hc % 2, :], func=Tanh,
                        bias=bp_sb[:, hc:hc + 1],
                    )
                    h1s.append(h1)
                h1_tiles[gi] = h1s

            def emit_l2l3(gi):
                start = gi * GS
                h1s = h1_tiles[gi]
                h2s = []
                for jc in range(2):
                    pj = pj_pool.tile([P, GS], f32, tag="pj",
                                      name=f"pj_{gi}_{jc}")
                    for kc in range(8):
                        nc.tensor.matmul(
                            out=pj[:],
                            lhsT=w2_sb[:, jc, kc, :],
                            rhs=h1s[kc][:],
                            start=(kc == 0),
                            stop=(kc == 7),
                        )
                    h2 = h2_pool.tile([P, GS], bf16, tag="h2",
                                      name=f"h2_{gi}_{jc}")
                    nc.scalar.activation(
                        out=h2[:], in_=pj[:], func=Tanh,
                        bias=bp_sb[:, 8 + jc:9 + jc],
                    )
                    h2s.append(h2)

                po = pt_pool.tile([NREL, GS], f32, tag="pt", name=f"po_{gi}")
                for kc in range(2):
                    nc.tensor.matmul(
                        out=po[:],
                        lhsT=w3_sb[:, kc, :],
                        rhs=h2s[kc][:],
                        start=(kc == 0),
                        stop=(kc == 1),
                    )
                o = out_pool.tile([NREL, GS], bf16, tag="o", name=f"o_{gi}")
                nc.vector.tensor_copy(out=o[:], in_=po[:])
                nc.sync.dma_start(outT[:, start:start + GS], o[:])

            # --- pipeline: L2/L3 lag one group ---
            emit_transpose(0)
            for gi in range(NG):
                emit_l1_a(gi)
                emit_l1_b(gi)
                if gi + 1 < NG:
                    emit_transpose(gi + 1)
                if gi >= 1:
                    emit_l2l3(gi - 1)
            emit_l2l3(NG - 1)

    nc.finalize()
    return nc


def _prep_inputs(inputs, rhidLayerFOH, rhidLayerFOM, rcatBias, rhid2Layer,
                 rhid2Bias, routLayer, routBias, heads):
    wdt = ml_dtypes.bfloat16
    inputs = np.asarray(inputs, dtype=np.float32)
    heads = np.asarray(heads)

    fwd = np.ascontiguousarray(inputs[:, 0, :]).astype(wdt)
    bwd_full = inputs[:, 1, :]
    mods_pad = np.concatenate(
        [np.arange(1, N_TOKENS), [N_TOKENS - 1]]).astype(np.int64)
    heads_pad = np.concatenate([heads.astype(np.int64), [0]]).astype(np.int32)

    w1 = np.ascontiguousarray(
        np.concatenate([np.asarray(rhidLayerFOH), np.asarray(rhidLayerFOM)],
                       axis=1)).astype(wdt)                      # [512, 1024]
    w2p = np.ascontiguousarray(
        np.asarray(rhid2Layer, dtype=np.float32)
        .reshape(8, P, 2, HID2 // 2).transpose(1, 2, 0, 3)).astype(wdt)
    w3p = np.ascontiguousarray(
        np.asarray(routLayer, dtype=np.float32)
        .reshape(2, P, NREL).transpose(1, 0, 2)).astype(wdt)
    b1 = np.asarray(rcatBias, dtype=np.float32).reshape(8, P).T
    b2 = np.asarray(rhid2Bias, dtype=np.float32).reshape(2, P).T
    bpack = np.ascontiguousarray(np.concatenate([b1, b2], axis=1))

    in_maps = []
    for c in range(NCORES):
        sl = slice(c * EPC, (c + 1) * EPC)
        bwd_c = bwd_full[mods_pad[sl]]                           # [2048, 256]
        bwdT_c = np.ascontiguousarray(
            bwd_c.T.reshape(2, P, EPC).transpose(1, 0, 2)).astype(wdt)
        headsT_c = np.ascontiguousarray(heads_pad[sl].reshape(SUB, P).T)
        in_maps.append({
            "fwd": fwd, "bwdT": bwdT_c, "headsT": headsT_c,
            "w1": w1, "w2p": w2p, "w3p": w3p, "bpack": bpack,
            "identw": np.eye(P, dtype=wdt),
        })
    return in_maps


def kernel(inputs, rhidLayerFOH, rhidLayerFOM, rcatBias, rhid2Layer, rhid2Bias,
           routLayer, routBias, heads):
    global LAST_RESULTS

    in_maps = _prep_inputs(inputs, rhidLayerFOH, rhidLayerFOM, rcatBias,
                           rhid2Layer, rhid2Bias, routLayer, routBias, heads)

    if "nc" not in _CACHE:
        _CACHE["nc"] = _build()
    nc = _CACHE["nc"]

    trace_dir = os.environ.get("KERNEL_TRACE_DIR") or None
    res = run_bass_kernel_spmd(nc, in_maps, list(range(NCORES)), tmpdir=trace_dir)
    LAST_RESULTS = res

    outT = np.concatenate(
        [np.asarray(r["outT"], dtype=np.float32) for r in res.results], axis=1)
    out = outT.T[:E] + np.asarray(routBias, dtype=np.float32)[None, :]
    return np.ascontiguousarray(out).astype(np.float32)
